# revision 1
# baseline (speedup 1.0000x reference)
"""SE(3)-CNN block (TensorProduct -> SE3Conv -> SE3BatchNorm -> BiasRelu) on 8 trn2 cores.

Sharding: core c = (batch b=c//2, out-x-half h=c%2). Each core computes all 64
output channels for 8 of 16 output x-planes of one batch; per-field BN second
moments are combined with a tiny [1,64] AllReduce across all 8 cores.

Conv strategy: the 9 t-channels per vector pair are symmetric (t = v (x) v), so
the 208 input channels reduce to 160 symmetrized ones. The contraction runs as
fp32r matmuls (measured ~1.1 cyc/row on trn2, ~8e-4 max rel err), one matmul
per (ky, kz, kx, out-x-plane) with free dim = (oy-range x full z). Slabs are
x-padded and z-padded with zeros so every matmul's free box is z-complete —
fp32r's walrus codegen rejects most partial-box access patterns. Chunk1
(channels 0:128) and chunk2 (channels 128:160, stored 4x with x-shifts so 4 kx
taps pack into one 128-row contraction) run as two phases sharing one slab
tile.
"""
import numpy as np
from itertools import product

# problem constants (from spec / reference)
B = 4
S_IN = 16
V_IN = 16
CO = 64          # 16 scalar + 48 vector output channels
CI = 160         # 16 s + 48 v + 96 t_sym
SIZE = 7
PAD = 3
STRIDE = 2
EPS = 1e-5
NCORES = 8
NXS = 22         # x-padded slab planes per core
NZS = 19         # z-padded: zi_slab = zi_global + 2, covering zofs in [-2, 1]
OXC = 8          # out x-planes per core
NQ = 4           # psum plane-pair banks
PAIRS = [(0, 0), (0, 1), (0, 2), (1, 1), (1, 2), (2, 2)]
VAR_S_DIV = 1.0 / (B * 16 * 16 * 16)
VAR_V_DIV = 1.0 / (B * 3 * 16 * 16 * 16)

SLAB_SHAPE = (128, NXS, 32, 2, NZS)   # [ci, px, iy, pz, zi]
WP_COLS = 8 * CO   # 512: 4 tap-pairs x 128
WA_COLS = 15 * CO  # 960: tap-pairs then 7 single-tap blocks
W2_COLS = 2 * CO   # 128
NSLOT = 9          # psum half-bank slots; slot s = (bank s//2, half s%2)


# ---------------------------------------------------------------- host prep

def _assemble_kernel_sym(inp):
    """Assemble the dense conv kernel [64, 208, 7,7,7] and symmetrize the
    t-block -> [64, 160, 7,7,7]."""
    def blk(w, basis):
        w = np.asarray(w, np.float32)
        basis = np.asarray(basis, np.float32)
        mo, mi, nb = w.shape
        do, di = basis.shape[1], basis.shape[2]
        k = np.einsum('uvb,bijxyz->uivjxyz', w, basis)
        return k.reshape(mo * do, mi * di, SIZE, SIZE, SIZE)

    row_s = np.concatenate([blk(inp['w_ss'], inp['basis_ss']),
                            blk(inp['w_sv'], inp['basis_sv']),
                            blk(inp['w_st'], inp['basis_st'])], axis=1)
    row_v = np.concatenate([blk(inp['w_vs'], inp['basis_vs']),
                            blk(inp['w_vv'], inp['basis_vv']),
                            blk(inp['w_vt'], inp['basis_vt'])], axis=1)
    K = np.concatenate([row_s, row_v], axis=0)  # [64, 208, 7,7,7]

    Ks = np.empty((CO, CI, SIZE, SIZE, SIZE), np.float32)
    Ks[:, :64] = K[:, :64]
    for u in range(16):
        for pi, (i, j) in enumerate(PAIRS):
            src = K[:, 64 + 9 * u + 3 * i + j]
            if i != j:
                src = src + K[:, 64 + 9 * u + 3 * j + i]
            Ks[:, 64 + 6 * u + pi] = src
    return Ks


def _svt_sym(sv):
    """[4,64,32,32,32] -> symmetrized tensor-product features [4,160,32,32,32]."""
    sv = np.asarray(sv, np.float32)
    s = sv[:, :S_IN]
    v = sv[:, S_IN:].reshape(B, V_IN, 3, 32, 32, 32)
    t = np.empty((B, V_IN, 6, 32, 32, 32), np.float32)
    for pi, (i, j) in enumerate(PAIRS):
        t[:, :, pi] = v[:, :, i] * v[:, :, j]
    return np.concatenate([s, v.reshape(B, 48, 32, 32, 32),
                           t.reshape(B, 96, 32, 32, 32)], axis=1)


def _core_slabs(svt, b, h):
    """x/z zero-padded, z-parity-split slabs for core (b, h).

    Returns (c1, c2), each SLAB_SHAPE float32. c1 plane px holds global
    ix = px + 16h - 3; c2 block a (rows 32a:32a+32) holds chunk-2 channels at
    ix = px + a + 16h - 3. zi_slab = zi_global + 2.
    """
    sp = svt[b].reshape(CI, 32, 32, 16, 2)   # (ci, x, y, zi, pz); iz = 2*zi + pz
    sp = np.moveaxis(sp, 4, 3)               # (ci, x, y, pz, zi)
    x0 = 16 * h - 3
    c1 = np.zeros(SLAB_SHAPE, np.float32)
    lo, hi = max(0, x0), min(32, x0 + NXS)
    c1[:, lo - x0:hi - x0, :, :, 2:18] = sp[:128, lo:hi]
    c2 = np.zeros(SLAB_SHAPE, np.float32)
    for a in range(4):
        sh = x0 + a
        lo2, hi2 = max(0, sh), min(32, sh + NXS)
        c2[32 * a:32 * a + 32, lo2 - sh:hi2 - sh, :, :, 2:18] = sp[128:160, lo2:hi2]
    return c1, c2


KX_PAIRS = [(0, 2), (4, 6), (1, 3), (3, 5)]


def _weight_slabs(Ks):
    """(WA1, WA2, W2): per-phase chunk-1 lhsT slabs (tap-pair and single-tap
    blocks, only those the phase uses) and the 4-way kx-merged chunk2 slab."""
    out = {}
    for phase in ('a',):
        cm, width = _CMAPS[phase]
        W = np.zeros((49, 128, width), np.float32)
        for ky, kz in product(range(SIZE), range(SIZE)):
            i = ky * SIZE + kz
            for blk, off in cm.items():
                if blk[0] == 'P':
                    ka, kb = blk[1]
                    W[i, :, off:off + 64] = Ks[:, :128, ka, ky, kz].T
                    W[i, :, off + 64:off + 128] = Ks[:, :128, kb, ky, kz].T
                else:
                    W[i, :, off:off + 64] = Ks[:, :128, blk[1], ky, kz].T
        out[phase] = W
    W2 = np.zeros((49, 128, W2_COLS), np.float32)
    for ky, kz in product(range(SIZE), range(SIZE)):
        i = ky * SIZE + kz
        for g in range(2):
            for a in range(4):
                kx = 4 * g + a
                if kx > 6:
                    continue
                W2[i, 32 * a:32 * a + 32, 64 * g:64 * (g + 1)] = \
                    Ks[:, 128:160, kx, ky, kz].T
    return out['a'], W2


def _gam_bias(bn_g_s, bn_g_v, bias_s):
    """Per-channel gamma [64] (vector gammas replicated x3) and bias [64]."""
    gam = np.empty(64, np.float32)
    gam[:16] = np.asarray(bn_g_s, np.float32)
    gam[16:] = np.repeat(np.asarray(bn_g_v, np.float32), 3)
    bias = np.zeros(64, np.float32)
    bias[:16] = np.asarray(bias_s, np.float32)
    return gam, bias


# ---------------------------------------------------------------- matmul plan

def _box(ky, kz):
    """Valid output range + slab coords for kernel offsets (ky, kz)."""
    d = kz - 3
    p = d % 2
    zofs = (d - p) // 2
    oy0 = max(0, (4 - ky) // 2)
    oy1 = min(16, (34 - ky) // 2 + 1)
    iy0 = 2 * oy0 + ky - 3
    return dict(p=p, zs=zofs + 2, iy0=iy0, oyc=oy1 - oy0, oy0=oy0)


def _phase_groups():
    groups = []
    for px in range(NXS):
        taps = [kx for kx in range(SIZE)
                if (px - kx) % 2 == 0 and 0 <= (px - kx) // 2 < OXC]
        pairs, used = [], set()
        for ka, kb in KX_PAIRS:
            if ka in taps and kb in taps and ka not in used and kb not in used:
                pairs.append((ka, kb))
                used.update((ka, kb))
        singles = [k for k in taps if k not in used]
        groups.append((pairs, singles))
    return groups


def _mm_plan():
    """Matmul descriptors (phase, i, kind, block, px, bx, slot) in issue order.

    Slots are psum half-banks (slot s = bank s//2, half s%2). Phase-A pairs
    write plane s at partitions 0:64 and plane s-1 at 64:128 of slot s;
    chunk-2 pairs use slots 9+(j-2): plane j lower, plane j-2 upper.

    Order: chunk-2 first (its half-size slab loads fast and its compute
    covers the big chunk-1 slab's DMA), then chunk-1.
    """
    groups = _phase_groups()

    def chunk1(phase, px_range):
        for ky, kz in product(range(SIZE), range(SIZE)):
            i = ky * SIZE + kz
            bx = _box(ky, kz)
            for px in px_range:
                pairs, singles = groups[px]
                for ka, kb in pairs:
                    yield (phase, i, 'pair', (ka, kb), px, bx, (px - ka) // 2)
                for k in singles:
                    yield (phase, i, 's1', k, px, bx, (px - k) // 2)

    def chunk2():
        for ky, kz in product(range(SIZE), range(SIZE)):
            i = ky * SIZE + kz
            bx = _box(ky, kz)
            for px in (0, 2):
                yield ('b', i, 'c2s', 0, px, bx, px // 2)
            for px in (4, 6, 8, 10, 12, 14):
                yield ('b', i, 'c2p', None, px, bx, 9 + px // 2 - 2)
            for px in (16, 18):
                yield ('b', i, 'c2s', 1, px, bx, (px - 4) // 2)

    yield from chunk2()
    yield from chunk1('a', range(NXS))


def _colmaps():
    """{('P', pair) | ('S', k) -> column offset} for the chunk-1 weight slab."""
    groups = _phase_groups()
    blocks = []
    for px in range(NXS):
        pairs, singles = groups[px]
        for pr in pairs:
            if ('P', pr) not in blocks:
                blocks.append(('P', pr))
        for k in singles:
            if ('S', k) not in blocks:
                blocks.append(('S', k))
    cm, off = {}, 0
    for b in blocks:
        cm[b] = off
        off += 128 if b[0] == 'P' else 64
    return {'a': (cm, off)}


_CMAPS = _colmaps()
WA_COLS = _CMAPS['a'][1]


# stop flags: last matmul covering each psum region (slot, L/U)
_LAST_IDX = {}
for _n, _d in enumerate(_mm_plan()):
    _kind, _s = _d[2], _d[6]
    _LAST_IDX[(_s, 'L')] = _n
    if _kind in ('pair', 'c2p'):
        _LAST_IDX[(_s, 'U')] = _n
_STOPS = set(_LAST_IDX.values())


# ---------------------------------------------------------------- numpy shadow

def _shadow_core(c1, c2, WA, W2):
    """Execute the matmul plan in numpy. Returns conv output [64, 8, 16, 16]."""
    accL = np.zeros((16, CO, 16, 16), np.float32)
    accU = np.zeros((16, CO, 16, 16), np.float32)
    WAs = {'a': WA}
    for phase, i, kind, blk, px, bx, s in _mm_plan():
        sl = c2 if kind in ('c2p', 'c2s') else c1
        rhs = sl[:, px, bx['iy0']:bx['iy0'] + 2 * bx['oyc']:2, bx['p'],
                 bx['zs']:bx['zs'] + 16]
        if kind == 'pair':
            off = _CMAPS[phase][0][('P', blk)]
            lhsT = WAs[phase][i][:, off:off + 128]
        elif kind == 's1':
            off = _CMAPS[phase][0][('S', blk)]
            lhsT = WAs[phase][i][:, off:off + 64]
        elif kind == 'c2p':
            lhsT = W2[i][:, 0:128]
        else:
            lhsT = W2[i][:, 64 * blk:64 * blk + 64]
        contrib = np.einsum('km,kbc->mbc', lhsT, rhs)
        ys = slice(bx['oy0'], bx['oy0'] + bx['oyc'])
        accL[s][:, ys, :] += contrib[:64]
        if kind in ('pair', 'c2p'):
            accU[s][:, ys, :] += contrib[64:]
    out = np.empty((OXC, CO, 16, 16), np.float32)
    for j in range(OXC):
        out[j] = accL[j] + accU[j + 1]
        if 2 <= j:
            out[j] += accL[9 + j - 2]
        if j <= 5:
            out[j] += accU[9 + j]
    return out.transpose(1, 0, 2, 3)


def shadow_forward(inp):
    """Full-model numpy shadow of the device computation (for plan validation)."""
    svt = _svt_sym(inp['sv'])
    Ks = _assemble_kernel_sym(inp)
    WA, W2 = _weight_slabs(Ks)
    gam, bias = _gam_bias(inp['bn_g_s'], inp['bn_g_v'], inp['bias_s'])

    y = np.zeros((B, CO, 16, 16, 16), np.float32)
    ss = np.zeros(64, np.float64)
    for c in range(NCORES):
        b, h = c // 2, c % 2
        c1, c2 = _core_slabs(svt, b, h)
        out = _shadow_core(c1, c2, WA, W2)
        y[b, :, 8 * h:8 * h + 8] = out
        ss += (out.astype(np.float64) ** 2).sum(axis=(1, 2, 3))

    var = np.empty(64)
    var[:16] = ss[:16] * VAR_S_DIV
    vv = (ss[16::3] + ss[17::3] + ss[18::3]) * VAR_V_DIV
    var[16:] = np.repeat(vv, 3)
    scale = gam / np.sqrt(var + EPS)
    y = y * scale[None, :, None, None, None].astype(np.float32)
    y[:, :16] = np.maximum(y[:, :16] + bias[:16][None, :, None, None, None], 0.0)
    return y


# ---------------------------------------------------------------- bass kernel

_CACHED = {}


def _build_bass():
    import concourse.bass as bass
    import concourse.tile as tile
    import concourse.mybir as mybir
    from concourse import bacc

    f32 = mybir.dt.float32
    f32r = mybir.dt.float32r

    nc = bacc.Bacc("TRN2", target_bir_lowering=False, debug=False, num_devices=NCORES)

    in1 = nc.dram_tensor("in1", list(SLAB_SHAPE), f32r, kind="ExternalInput").ap()
    in2e = nc.dram_tensor("in2e", [128, 11, 32, 2, NZS], f32r, kind="ExternalInput").ap()
    wa_in = nc.dram_tensor("wa_in", [49, 128, WA_COLS], f32r, kind="ExternalInput").ap()
    w2_in = nc.dram_tensor("w2_in", [49, 128, W2_COLS], f32r, kind="ExternalInput").ap()
    gam_in = nc.dram_tensor("gam_in", [64, 1], f32, kind="ExternalInput").ap()
    bias_in = nc.dram_tensor("bias_in", [64, 1], f32, kind="ExternalInput").ap()
    out_d = nc.dram_tensor("out", [CO, OXC, 16, 16], f32, kind="ExternalOutput").ap()

    plan = list(_mm_plan())

    with tile.TileContext(nc) as tc:
        with (
            tc.tile_pool(name="slab", bufs=1) as slab_pool,
            tc.tile_pool(name="wp", bufs=3) as wpool,
            tc.tile_pool(name="ps", bufs=1, space="PSUM") as ps,
            tc.tile_pool(name="outp", bufs=1) as outp,
            tc.tile_pool(name="stat", bufs=1) as stat,
            tc.tile_pool(name="dram", bufs=1, space="DRAM") as dram,
        ):
            # 8 psum banks = 16 half-bank slots (15 used)
            pq = [ps.tile([128, 2, 16, 16], f32, tag=f"pq{t}", name=f"pq{t}")
                  for t in range(8)]

            def slot_ap(s, lo, hi, oy0, oyc):
                return pq[s // 2][lo:hi, s % 2, oy0:oy0 + oyc, :]

            # load order matches compute order: chunk-1 planes 0..10, then
            # the chunk-2 half slab, then chunk-1 planes 11..21
            sl1 = slab_pool.tile(list(SLAB_SHAPE), f32r, tag="slab", name="slab_c1")
            sl2 = slab_pool.tile([128, 11, 32, 2, NZS], f32r, tag="slab2",
                                 name="slab_c2")
            for xi in range(11):
                nc.sync.dma_start(sl2[:, xi], in2e[:, xi])
            for px in range(NXS):
                nc.sync.dma_start(sl1[:, px], in1[:, px])

            # start=True clears the WHOLE psum bank, so open each bank once
            # with a zero-weight full-bank matmul (also a WAW dep that orders
            # it before every accumulate); all real matmuls use start=False.
            zw_f = stat.tile([128, 128], f32, tag="zw")
            nc.vector.memset(zw_f[:], 0.0)
            zw = zw_f.bitcast(f32r)
            zrhs = sl2[:, 0, 0:32, 0, 0:16]
            for t in range(8):
                nc.tensor.matmul(pq[t].rearrange("c a y z -> c (a y z)"),
                                 zw[:], zrhs, start=True, stop=False)

            w_dram = {'a': wa_in, 'b': w2_in}
            w_cols = {'a': WA_COLS, 'b': W2_COLS}

            def emit(descs, sl, base, halfx):
                phase0 = descs[0][0]
                wtile = [None]
                cur_i = [-1]
                for n, (phase, i, kind, blk, px, bx, s) in enumerate(descs):
                    if i != cur_i[0]:
                        w = wpool.tile([128, w_cols[phase]], f32r, tag="w",
                                       name=f"w_{phase}_{i}", bufs=6)
                        for pc in range(4):
                            nc.gpsimd.dma_start(w[32 * pc:32 * pc + 32],
                                                w_dram[phase][i, 32 * pc:32 * pc + 32])
                        wtile[0] = w
                        cur_i[0] = i
                    w = wtile[0]
                    if kind == 'pair':
                        wc, ww = _CMAPS[phase][0][('P', blk)], 128
                    elif kind == 's1':
                        wc, ww = _CMAPS[phase][0][('S', blk)], 64
                    elif kind == 'c2p':
                        wc, ww = 0, 128
                    else:
                        wc, ww = 64 * blk, 64
                    xidx = px // 2 if halfx else px
                    rhs = sl[:, xidx, bx['iy0']:bx['iy0'] + 2 * bx['oyc'] - 1:2,
                             bx['p'], bx['zs']:bx['zs'] + 16]
                    gn = n + base
                    stop = gn in _STOPS
                    out_ap = slot_ap(s, 0, ww, bx['oy0'], bx['oyc'])
                    nc.tensor.matmul(out_ap, w[:, wc:wc + ww], rhs,
                                     start=False, stop=stop)

            n_b = sum(1 for d in plan if d[0] == 'b')
            emit(plan[:n_b], sl2, 0, True)
            emit(plan[n_b:], sl1, n_b, False)

            # evacuate: plane j = L(slot j) + U(slot j+1)
            #                    [+ LB(slot 9+j-2) j>=2] [+ UB(slot 9+j) j<=5]
            osb = outp.tile([CO, OXC, 16, 16], f32, tag="osb")
            usb = outp.tile([128, OXC, 16, 16], f32, tag="usb")
            for j in range(OXC):
                nc.vector.tensor_copy(osb[:, j], pq[j // 2][0:64, j % 2])
                if j >= 2:
                    s = 9 + j - 2
                    nc.vector.tensor_add(osb[:, j], osb[:, j],
                                         pq[s // 2][0:64, s % 2])
                su = j + 1
                nc.vector.tensor_copy(usb[64:128, j], pq[su // 2][64:128, su % 2])
                if j <= 5:
                    s = 9 + j
                    nc.vector.tensor_add(usb[64:128, j], usb[64:128, j],
                                         pq[s // 2][64:128, s % 2])
            # move upper-half partials down to partitions 0:64 and add
            u_dram = dram.tile([64, OXC, 16, 16], f32, tag="ud")
            nc.sync.dma_start(u_dram[:], usb[64:128])
            nc.sync.dma_start(usb[0:64], u_dram[:])
            of = osb.rearrange("c x y z -> c (x y z)")
            uf = usb.rearrange("c x y z -> c (x y z)")
            nc.vector.tensor_add(of[:, :], of[:, :], uf[0:64, :])

            # per-channel sum of squares -> local variance contribution
            # (linear in the sums, so the AllReduce can carry variance
            # directly and the post-collective chain stays short)
            sq = outp.tile([CO, 2048], f32, tag="sq")
            ssq = stat.tile([CO, 1], f32, tag="ssq")
            nc.vector.tensor_mul(sq[:], of[:, :], of[:, :])
            nc.vector.tensor_reduce(ssq[:], sq[:], axis=mybir.AxisListType.X,
                                    op=mybir.AluOpType.add)
            ss_row = stat.tile([1, 64], f32, tag="ssrow")
            vloc = stat.tile([1, 64], f32, tag="vloc")
            tmp16 = stat.tile([1, 16], f32, tag="tmp16")
            ss_dram = dram.tile([1, 64], f32, tag="ssd")
            nc.gpsimd.dma_start(ss_dram[0, :], ssq[:, 0])
            nc.gpsimd.dma_start(ss_row[:], ss_dram[:])
            nc.vector.tensor_add(tmp16[:], ss_row[:, 16::3], ss_row[:, 17::3])
            nc.vector.tensor_add(tmp16[:], tmp16[:], ss_row[:, 18::3])
            nc.vector.tensor_scalar_mul(vloc[:, 0:16], ss_row[:, 0:16], VAR_S_DIV)
            for j in range(3):
                nc.vector.tensor_scalar_mul(vloc[:, 16 + j::3], tmp16[:], VAR_V_DIV)

            v_dram = dram.tile([1, 64], f32, tag="vd")
            v_red = dram.tile([1, 64], f32, tag="vr")
            nc.gpsimd.dma_start(v_dram[:], vloc[:])
            nc.gpsimd.collective_compute(
                "AllReduce", mybir.AluOpType.add,
                replica_groups=[list(range(NCORES))],
                ins=[v_dram.opt()], outs=[v_red.opt()],
            )

            # scale = gamma / sqrt(var + eps), in per-partition layout
            var_col = stat.tile([CO, 1], f32, tag="varcol")
            nc.gpsimd.dma_start(var_col[:, 0], v_red[0, :])
            eps_t = stat.tile([CO, 1], f32, tag="eps")
            nc.vector.memset(eps_t[:], EPS)
            sd = stat.tile([CO, 1], f32, tag="sd")
            nc.scalar.activation(sd[:], var_col[:], mybir.ActivationFunctionType.Sqrt,
                                 bias=eps_t[:], scale=1.0)
            inv = stat.tile([CO, 1], f32, tag="inv")
            nc.vector.reciprocal(inv[:], sd[:])
            gam_t = stat.tile([CO, 1], f32, tag="gam")
            nc.sync.dma_start(gam_t[:], gam_in[:])
            scale_col = stat.tile([CO, 1], f32, tag="sccol")
            nc.vector.tensor_mul(scale_col[:], inv[:], gam_t[:])
            bias_t = stat.tile([CO, 1], f32, tag="bias")
            nc.sync.dma_start(bias_t[:], bias_in[:])

            # apply BN scale everywhere, then bias+relu on scalar channels
            nc.vector.tensor_scalar_mul(of[:, :], of[:, :], scale_col[:, :])
            nc.scalar.activation(of[0:16, :], of[0:16, :],
                                 mybir.ActivationFunctionType.Relu,
                                 bias=bias_t[0:16, :], scale=1.0)
            nc.sync.dma_start(out_d[:], osb[:])

    nc.compile()
    return nc


def _install_ntff_hook():
    import sys, types
    if "antenv.axon_hooks" in sys.modules:
        return
    mod = types.ModuleType("antenv.axon_hooks")
    mod._hook = None
    mod.set_axon_ntff_profile_hook = lambda h: setattr(mod, "_hook", h)
    mod.get_axon_ntff_profile_hook = lambda: mod._hook
    sys.modules["antenv.axon_hooks"] = mod
    try:
        import antenv
        antenv.axon_hooks = mod
        from trn_agent_boot.trn_boot import _ntff_profile_via_ctypes
        mod.set_axon_ntff_profile_hook(_ntff_profile_via_ctypes("/opt/axon/libaxon_pjrt.so"))
    except Exception:
        pass


def run_on_hw(inp, trace=False):
    """Run the kernel on 8 cores. Returns (full output [4,64,16,16,16], results)."""
    from concourse.bass_utils import run_bass_kernel_spmd

    if "nc" not in _CACHED:
        _install_ntff_hook()
        _CACHED["nc"] = _build_bass()
    nc = _CACHED["nc"]

    svt = _svt_sym(inp['sv'])
    Ks = _assemble_kernel_sym(inp)
    WA, W2 = _weight_slabs(Ks)
    gam, bias = _gam_bias(inp['bn_g_s'], inp['bn_g_v'], inp['bias_s'])

    in_maps = []
    for c in range(NCORES):
        b, h = c // 2, c % 2
        c1, c2 = _core_slabs(svt, b, h)
        in_maps.append({
            "in1": c1,
            "in2e": np.ascontiguousarray(c2[:, ::2]),
            "wa_in": WA, "w2_in": W2,
            "gam_in": gam.reshape(64, 1),
            "bias_in": bias.reshape(64, 1),
        })

    res = run_bass_kernel_spmd(nc, in_maps, core_ids=list(range(NCORES)), trace=trace)

    y = np.zeros((B, CO, 16, 16, 16), np.float32)
    for c in range(NCORES):
        b, h = c // 2, c % 2
        y[b, :, 8 * h:8 * h + 8] = res.results[c]["out"]
    return y, res


def kernel(**inputs) -> np.ndarray:
    y, _ = run_on_hw(inputs, trace=False)
    return y



# revision 4
# speedup vs baseline: 1.1423x; 1.1423x over previous
"""SE(3)-CNN block (TensorProduct -> SE3Conv -> SE3BatchNorm -> BiasRelu) on 8 trn2 cores.

Sharding: core c = (batch b=c//2, out-x-half h=c%2). Each core computes all 64
output channels for 8 of 16 output x-planes of one batch; per-field BN second
moments are combined with a tiny [1,64] AllReduce across all 8 cores.

v2 conv strategy (vs single-plane baseline):
- Every fp32r matmul covers TWO output x-planes = one full psum bank, free
  dim 448-512 (>= 256 keeps fp32r in its fast streaming mode) and half the
  instruction count.
- No M=64 boundary singles: each chunk-1 kx pair block runs its full slot
  range (s=0..9); out-of-range tap contributions land in psum half-banks the
  evacuation never reads (slot 8 L, slot 9) fed from x-padded slab planes.
- chunk2 (channels 128:160, 4 kx taps packed per 128-row contraction) uses
  slots 10-15 (banks 5-7) plus L(0),L(1),L(6),L(7).
- Weight slab dedup: chunk-1 kx blocks stored once in column order
  [0,2 | 4,6 | 1,3 | 5] so every used pair is contiguous (11.2MB vs 24MB).
- Weight DMA on the Activation HWDGE queue, slabs + tail/stat DMAs on SP,
  evacuation adds split across Vector and GpSimd.
"""
import numpy as np
from itertools import product

# problem constants (from spec / reference)
B = 4
S_IN = 16
V_IN = 16
CO = 64          # 16 scalar + 48 vector output channels
CI = 160         # 16 s + 48 v + 96 t_sym
SIZE = 7
PAD = 3
STRIDE = 2
EPS = 1e-5
NCORES = 8
NXS = 23         # x-padded slab planes per core (px 22 feeds garbage slots only)
NZS = 19         # z-padded: zi_slab = zi_global + 2, covering zofs in [-2, 1]
NX2 = 10         # chunk-2 even slab planes (px = 2*xi, xi 0..9)
OXC = 8          # out x-planes per core
PAIRS = [(0, 0), (0, 1), (0, 2), (1, 1), (1, 2), (2, 2)]
VAR_S_DIV = 1.0 / (B * 16 * 16 * 16)
VAR_V_DIV = 1.0 / (B * 3 * 16 * 16 * 16)

SLAB1_SHAPE = (128, NXS, 32, 2, NZS)   # [ci, px, iy, pz, zi]
SLAB2_SHAPE = (128, NX2, 32, 2, NZS)
WA_COLS = 448    # [k0|k2 | k4|k6 | k1|k3 | k5]
W2_COLS = 2 * CO


# ---------------------------------------------------------------- host prep

def _assemble_kernel_sym(inp):
    """Assemble the dense conv kernel [64, 208, 7,7,7] and symmetrize the
    t-block -> [64, 160, 7,7,7]."""
    def blk(w, basis):
        w = np.asarray(w, np.float32)
        basis = np.asarray(basis, np.float32)
        mo, mi, nb = w.shape
        do, di = basis.shape[1], basis.shape[2]
        k = np.einsum('uvb,bijxyz->uivjxyz', w, basis)
        return k.reshape(mo * do, mi * di, SIZE, SIZE, SIZE)

    row_s = np.concatenate([blk(inp['w_ss'], inp['basis_ss']),
                            blk(inp['w_sv'], inp['basis_sv']),
                            blk(inp['w_st'], inp['basis_st'])], axis=1)
    row_v = np.concatenate([blk(inp['w_vs'], inp['basis_vs']),
                            blk(inp['w_vv'], inp['basis_vv']),
                            blk(inp['w_vt'], inp['basis_vt'])], axis=1)
    K = np.concatenate([row_s, row_v], axis=0)  # [64, 208, 7,7,7]

    Ks = np.empty((CO, CI, SIZE, SIZE, SIZE), np.float32)
    Ks[:, :64] = K[:, :64]
    for u in range(16):
        for pi, (i, j) in enumerate(PAIRS):
            src = K[:, 64 + 9 * u + 3 * i + j]
            if i != j:
                src = src + K[:, 64 + 9 * u + 3 * j + i]
            Ks[:, 64 + 6 * u + pi] = src
    return Ks


def _svt_sym(sv):
    """[4,64,32,32,32] -> symmetrized tensor-product features [4,160,32,32,32]."""
    sv = np.asarray(sv, np.float32)
    s = sv[:, :S_IN]
    v = sv[:, S_IN:].reshape(B, V_IN, 3, 32, 32, 32)
    t = np.empty((B, V_IN, 6, 32, 32, 32), np.float32)
    for pi, (i, j) in enumerate(PAIRS):
        t[:, :, pi] = v[:, :, i] * v[:, :, j]
    return np.concatenate([s, v.reshape(B, 48, 32, 32, 32),
                           t.reshape(B, 96, 32, 32, 32)], axis=1)


def _core_slabs(svt, b, h):
    """x/z zero-padded, z-parity-split slabs for core (b, h).

    c1 [128, 23, 32, 2, 19]: plane px holds global ix = px + 16h - 3.
    c2e [128, 10, 32, 2, 19]: block a (rows 32a:32a+32) of plane xi holds
    chunk-2 channels at ix = 2*xi + a + 16h - 3. zi_slab = zi_global + 2.
    """
    sp = svt[b].reshape(CI, 32, 32, 16, 2)   # (ci, x, y, zi, pz); iz = 2*zi + pz
    sp = np.moveaxis(sp, 4, 3)               # (ci, x, y, pz, zi)
    x0 = 16 * h - 3
    c1 = np.zeros(SLAB1_SHAPE, np.float32)
    lo, hi = max(0, x0), min(32, x0 + NXS)
    c1[:, lo - x0:hi - x0, :, :, 2:18] = sp[:128, lo:hi]
    c2e = np.zeros(SLAB2_SHAPE, np.float32)
    for a in range(4):
        for xi in range(NX2):
            ix = 2 * xi + a + x0
            if 0 <= ix < 32:
                c2e[32 * a:32 * a + 32, xi, :, :, 2:18] = sp[128:160, ix]
    return c1, c2e


def _weight_slabs(Ks):
    """(WA, W2). WA [49, 128, 448]: chunk-1 taps in column order
    [k0|k2|k4|k6|k1|k3|k5] (64 cols each); pairs (0,2),(4,6),(1,3) are the
    contiguous 128-col windows at 0, 128, 256; k5 singles at 384.
    W2 [49, 128, 128]: 4-way kx-merged chunk-2 (g=0: kx 0..3 lower 64 cols,
    g=1: kx 4..6 upper)."""
    KXORD = [0, 2, 4, 6, 1, 3, 5]
    WA = np.zeros((49, 128, WA_COLS), np.float32)
    W2 = np.zeros((49, 128, W2_COLS), np.float32)
    for ky, kz in product(range(SIZE), range(SIZE)):
        i = ky * SIZE + kz
        for ci, kx in enumerate(KXORD):
            WA[i, :, 64 * ci:64 * ci + 64] = Ks[:, :128, kx, ky, kz].T
        for g in range(2):
            for a in range(4):
                kx = 4 * g + a
                if kx > 6:
                    continue
                W2[i, 32 * a:32 * a + 32, 64 * g:64 * (g + 1)] = \
                    Ks[:, 128:160, kx, ky, kz].T
    return WA, W2


def _gam_bias(bn_g_s, bn_g_v, bias_s):
    """Per-channel gamma [64] (vector gammas replicated x3) and bias [64]."""
    gam = np.empty(64, np.float32)
    gam[:16] = np.asarray(bn_g_s, np.float32)
    gam[16:] = np.repeat(np.asarray(bn_g_v, np.float32), 3)
    bias = np.zeros(64, np.float32)
    bias[:16] = np.asarray(bias_s, np.float32)
    return gam, bias


# ---------------------------------------------------------------- matmul plan

def _box(ky, kz):
    """Valid output range + slab coords for kernel offsets (ky, kz)."""
    d = kz - 3
    p = d % 2
    zofs = (d - p) // 2
    oy0 = max(0, (4 - ky) // 2)
    oy1 = min(16, (34 - ky) // 2 + 1)
    iy0 = 2 * oy0 + ky - 3
    return dict(p=p, zs=zofs + 2, iy0=iy0, oyc=oy1 - oy0, oy0=oy0)


def _mm_plan():
    """Matmul descriptors (src, i, wc, ww, x0, bank) in issue order.

    Each matmul writes one full psum bank `bank` = slots (2*bank, 2*bank+1):
    partitions 0:ww x 2 halves x [oy0:oy0+oyc] x 16. Slot s holds plane s at
    partitions 0:64 (L) and plane s-1 at 64:128 (U). Chunk-2 c2p uses slots
    10..15 (banks 5-7): slot 10+j-2 L = plane j lower-taps, slot 10+j U =
    plane j upper-taps. Slot 8 L / slot 9 / U(0) are garbage sinks.

    Order: chunk-2 first (its half-size slab loads fast and its compute
    covers the big chunk-1 slab's DMA), then chunk-1.
    """
    plan = []
    for i in range(49):
        plan.append(('c2', i, 0, 64, 0, 0))     # c2s g0 -> L(0),L(1)
        plan.append(('c2', i, 64, 64, 8, 3))    # c2s g1 -> L(6),L(7)
        plan.append(('c2', i, 0, 128, 2, 5))    # c2p -> slots (10,11)
        plan.append(('c2', i, 0, 128, 4, 6))    # slots (12,13)
        plan.append(('c2', i, 0, 128, 6, 7))    # slots (14,15)
    for i in range(49):
        for bank in range(5):                   # P(0,2): px (4b, 4b+2)
            plan.append(('c1', i, 0, 128, 4 * bank, bank))
        for bank in range(5):                   # P(4,6): px (4+4b, 6+4b)
            plan.append(('c1', i, 128, 128, 4 + 4 * bank, bank))
        for bank in range(5):                   # P(1,3): px (1+4b, 3+4b)
            plan.append(('c1', i, 256, 128, 1 + 4 * bank, bank))
        for bank in range(4):                   # S5: px (5+4b, 7+4b), L only
            plan.append(('c1', i, 384, 64, 5 + 4 * bank, bank))
    return plan


_PLAN = _mm_plan()

# stop flags: last matmul touching each psum bank
_LAST_IDX = {}
for _n, _d in enumerate(_PLAN):
    _LAST_IDX[_d[5]] = _n
_STOPS = set(_LAST_IDX.values())


# ---------------------------------------------------------------- numpy shadow

def _shadow_core(c1, c2e, WA, W2):
    """Execute the matmul plan in numpy. Returns conv output [64, 8, 16, 16]."""
    banks = np.zeros((8, 128, 2, 16, 16), np.float32)
    for src, i, wc, ww, x0, bank in _PLAN:
        ky, kz = i // 7, i % 7
        bx = _box(ky, kz)
        sl = c2e if src == 'c2' else c1
        step = 1 if src == 'c2' else 2
        rhs = sl[:, x0:x0 + step + 1:step,
                 bx['iy0']:bx['iy0'] + 2 * bx['oyc']:2, bx['p'],
                 bx['zs']:bx['zs'] + 16]
        lhsT = (W2 if src == 'c2' else WA)[i][:, wc:wc + ww]
        contrib = np.einsum('km,kpbc->mpbc', lhsT, rhs)
        banks[bank][:ww, :, bx['oy0']:bx['oy0'] + bx['oyc'], :] += contrib
    out = np.empty((OXC, CO, 16, 16), np.float32)
    for j in range(OXC):
        acc = banks[j // 2][0:64, j % 2].copy()        # L(j)
        su = j + 1
        acc = acc + banks[su // 2][64:128, su % 2]     # U(j+1)
        if j >= 2:
            s = 8 + j                                  # LB: slot 10+j-2
            acc = acc + banks[s // 2][0:64, s % 2]
        if j <= 5:
            s = 10 + j                                 # UB: slot 10+j
            acc = acc + banks[s // 2][64:128, s % 2]
        out[j] = acc
    return out.transpose(1, 0, 2, 3)


def shadow_forward(inp):
    """Full-model numpy shadow of the device computation (for plan validation)."""
    svt = _svt_sym(inp['sv'])
    Ks = _assemble_kernel_sym(inp)
    WA, W2 = _weight_slabs(Ks)
    gam, bias = _gam_bias(inp['bn_g_s'], inp['bn_g_v'], inp['bias_s'])

    y = np.zeros((B, CO, 16, 16, 16), np.float32)
    ss = np.zeros(64, np.float64)
    for c in range(NCORES):
        b, h = c // 2, c % 2
        c1, c2e = _core_slabs(svt, b, h)
        out = _shadow_core(c1, c2e, WA, W2)
        y[b, :, 8 * h:8 * h + 8] = out
        ss += (out.astype(np.float64) ** 2).sum(axis=(1, 2, 3))

    var = np.empty(64)
    var[:16] = ss[:16] * VAR_S_DIV
    vv = (ss[16::3] + ss[17::3] + ss[18::3]) * VAR_V_DIV
    var[16:] = np.repeat(vv, 3)
    scale = gam / np.sqrt(var + EPS)
    y = y * scale[None, :, None, None, None].astype(np.float32)
    y[:, :16] = np.maximum(y[:, :16] + bias[:16][None, :, None, None, None], 0.0)
    return y


# ---------------------------------------------------------------- bass kernel

_CACHED = {}


def _build_bass():
    import concourse.bass as bass
    import concourse.tile as tile
    import concourse.mybir as mybir
    from concourse import bacc

    f32 = mybir.dt.float32
    f32r = mybir.dt.float32r

    nc = bacc.Bacc("TRN2", target_bir_lowering=False, debug=False, num_devices=NCORES)

    in1 = nc.dram_tensor("in1", list(SLAB1_SHAPE), f32r, kind="ExternalInput").ap()
    in2e = nc.dram_tensor("in2e", list(SLAB2_SHAPE), f32r, kind="ExternalInput").ap()
    wa_in = nc.dram_tensor("wa_in", [49, 128, WA_COLS], f32r, kind="ExternalInput").ap()
    w2_in = nc.dram_tensor("w2_in", [49, 128, W2_COLS], f32r, kind="ExternalInput").ap()
    gam_in = nc.dram_tensor("gam_in", [64, 1], f32, kind="ExternalInput").ap()
    bias_in = nc.dram_tensor("bias_in", [64, 1], f32, kind="ExternalInput").ap()
    out_d = nc.dram_tensor("out", [CO, OXC, 16, 16], f32, kind="ExternalOutput").ap()

    with tile.TileContext(nc) as tc:
        with (
            tc.tile_pool(name="slab", bufs=1) as slab_pool,
            tc.tile_pool(name="wp", bufs=4) as wpool,
            tc.tile_pool(name="ps", bufs=1, space="PSUM") as ps,
            tc.tile_pool(name="outp", bufs=1) as outp,
            tc.tile_pool(name="stat", bufs=1) as stat,
            tc.tile_pool(name="dram", bufs=1, space="DRAM") as dram,
        ):
            # 8 psum banks = 16 half-bank slots (garbage: 8L, 9, U(0))
            pq = [ps.tile([128, 2, 16, 16], f32, tag=f"pq{t}", name=f"pq{t}")
                  for t in range(8)]

            # tiny BN params first on the SP queue so they're resident early
            gam_t = stat.tile([CO, 1], f32, tag="gam")
            bias_t = stat.tile([CO, 1], f32, tag="bias")
            nc.sync.dma_start(gam_t[:], gam_in[:])
            nc.sync.dma_start(bias_t[:], bias_in[:])

            # slabs on SP HWDGE: chunk-2 planes needed first, then chunk-1
            sl2 = slab_pool.tile(list(SLAB2_SHAPE), f32r, tag="slab2",
                                 name="slab_c2")
            sl1 = slab_pool.tile(list(SLAB1_SHAPE), f32r, tag="slab",
                                 name="slab_c1")
            nc.sync.dma_start(sl2[:, 0:2], in2e[:, 0:2])
            nc.sync.dma_start(sl2[:, 2:8], in2e[:, 2:8])
            nc.sync.dma_start(sl2[:, 8:10], in2e[:, 8:10])
            nc.sync.dma_start(sl1[:, 0:12], in1[:, 0:12])
            nc.sync.dma_start(sl1[:, 12:NXS], in1[:, 12:NXS])

            # start=True clears the WHOLE psum bank, so open each bank once
            # with a zero-weight full-bank matmul (also a WAW dep that orders
            # it before every accumulate); all real matmuls use start=False.
            # rhs is a memset zeros tile so the opens run before any DMA lands.
            zw_f = stat.tile([128, 128], f32, tag="zw")
            zr_f = stat.tile([128, 512], f32, tag="zr")
            nc.vector.memset(zw_f[:], 0.0)
            nc.vector.memset(zr_f[:], 0.0)
            zw = zw_f.bitcast(f32r)
            zr = zr_f.bitcast(f32r)
            for t in range(8):
                nc.tensor.matmul(pq[t].rearrange("c a y z -> c (a y z)"),
                                 zw[:], zr[:], start=True, stop=False)

            # weights on the Activation HWDGE queue, one DMA per tile
            w2t = {}
            for i in range(49):
                w = wpool.tile([128, W2_COLS], f32r, tag="w2", name=f"w2_{i}",
                               bufs=4)
                nc.scalar.dma_start(w[:], w2_in[i])
                w2t[i] = w
            wat = {}
            for i in range(49):
                w = wpool.tile([128, WA_COLS], f32r, tag="wa", name=f"wa_{i}",
                               bufs=4)
                nc.scalar.dma_start(w[:], wa_in[i])
                wat[i] = w

            for n, (src, i, wc, ww, x0, bank) in enumerate(_PLAN):
                ky, kz = i // 7, i % 7
                bx = _box(ky, kz)
                if src == 'c2':
                    w, sl, step = w2t[i], sl2, 1
                else:
                    w, sl, step = wat[i], sl1, 2
                rhs = sl[:, x0:x0 + step + 1:step,
                         bx['iy0']:bx['iy0'] + 2 * bx['oyc'] - 1:2, bx['p'],
                         bx['zs']:bx['zs'] + 16]
                out_ap = pq[bank][0:ww, 0:2, bx['oy0']:bx['oy0'] + bx['oyc'], :]
                nc.tensor.matmul(out_ap, w[:, wc:wc + ww], rhs,
                                 start=False, stop=n in _STOPS)

            # evacuate: plane j = L(j) + U(j+1) [+ LB(10+j-2) j>=2]
            #                                   [+ UB(10+j)   j<=5]
            # a TensorTensor may read only ONE psum operand: Scalar engine
            # copies psum->sbuf, Vector accumulates the second psum operand
            osb = outp.tile([CO, OXC, 16, 16], f32, tag="osb")
            usb = outp.tile([128, OXC, 16, 16], f32, tag="usb")
            for j in range(OXC):
                nc.scalar.activation(osb[:, j], pq[j // 2][0:64, j % 2],
                                     mybir.ActivationFunctionType.Copy,
                                     scale=1.0)
                if j >= 2:
                    s = 8 + j
                    nc.vector.tensor_add(osb[:, j], osb[:, j],
                                         pq[s // 2][0:64, s % 2])
                su = j + 1
                nc.scalar.activation(usb[64:128, j],
                                     pq[su // 2][64:128, su % 2],
                                     mybir.ActivationFunctionType.Copy,
                                     scale=1.0)
                if j <= 5:
                    s = 10 + j
                    nc.vector.tensor_add(usb[64:128, j], usb[64:128, j],
                                         pq[s // 2][64:128, s % 2])
            # move upper-half partials down to partitions 0:64 and add
            u_dram = dram.tile([64, OXC, 16, 16], f32, tag="ud")
            nc.sync.dma_start(u_dram[:], usb[64:128])
            nc.sync.dma_start(usb[0:64], u_dram[:])
            of = osb.rearrange("c x y z -> c (x y z)")
            uf = usb.rearrange("c x y z -> c (x y z)")
            nc.vector.tensor_add(of[:, :], of[:, :], uf[0:64, :])

            # per-channel sum of squares -> local variance contribution
            # (linear in the sums, so the AllReduce can carry variance
            # directly and the post-collective chain stays short)
            sq = outp.tile([CO, 2048], f32, tag="sq")
            ssq = stat.tile([CO, 1], f32, tag="ssq")
            nc.vector.tensor_mul(sq[:], of[:, :], of[:, :])
            nc.vector.tensor_reduce(ssq[:], sq[:], axis=mybir.AxisListType.X,
                                    op=mybir.AluOpType.add)
            ss_row = stat.tile([1, 64], f32, tag="ssrow")
            vloc = stat.tile([1, 64], f32, tag="vloc")
            tmp16 = stat.tile([1, 16], f32, tag="tmp16")
            ss_dram = dram.tile([1, 64], f32, tag="ssd")
            nc.sync.dma_start(ss_dram[0, :], ssq[:, 0])
            nc.sync.dma_start(ss_row[:], ss_dram[:])
            nc.vector.tensor_add(tmp16[:], ss_row[:, 16::3], ss_row[:, 17::3])
            nc.vector.tensor_add(tmp16[:], tmp16[:], ss_row[:, 18::3])
            nc.vector.tensor_scalar_mul(vloc[:, 0:16], ss_row[:, 0:16], VAR_S_DIV)
            for j in range(3):
                nc.vector.tensor_scalar_mul(vloc[:, 16 + j::3], tmp16[:], VAR_V_DIV)

            v_dram = dram.tile([1, 64], f32, tag="vd")
            v_red = dram.tile([1, 64], f32, tag="vr")
            nc.sync.dma_start(v_dram[:], vloc[:])
            nc.gpsimd.collective_compute(
                "AllReduce", mybir.AluOpType.add,
                replica_groups=[list(range(NCORES))],
                ins=[v_dram.opt()], outs=[v_red.opt()],
            )

            # scale = gamma / sqrt(var + eps), in per-partition layout
            var_col = stat.tile([CO, 1], f32, tag="varcol")
            nc.sync.dma_start(var_col[:, 0], v_red[0, :])
            eps_t = stat.tile([CO, 1], f32, tag="eps")
            nc.vector.memset(eps_t[:], EPS)
            sd = stat.tile([CO, 1], f32, tag="sd")
            nc.scalar.activation(sd[:], var_col[:], mybir.ActivationFunctionType.Sqrt,
                                 bias=eps_t[:], scale=1.0)
            inv = stat.tile([CO, 1], f32, tag="inv")
            nc.vector.reciprocal(inv[:], sd[:])
            scale_col = stat.tile([CO, 1], f32, tag="sccol")
            nc.vector.tensor_mul(scale_col[:], inv[:], gam_t[:])

            # apply BN scale everywhere, then bias+relu on scalar channels
            nc.vector.tensor_scalar_mul(of[:, :], of[:, :], scale_col[:, :])
            nc.scalar.activation(of[0:16, :], of[0:16, :],
                                 mybir.ActivationFunctionType.Relu,
                                 bias=bias_t[0:16, :], scale=1.0)
            nc.sync.dma_start(out_d[:], osb[:])

    nc.compile()
    return nc


def _install_ntff_hook():
    import sys, types
    if "antenv.axon_hooks" in sys.modules:
        return
    mod = types.ModuleType("antenv.axon_hooks")
    mod._hook = None
    mod.set_axon_ntff_profile_hook = lambda h: setattr(mod, "_hook", h)
    mod.get_axon_ntff_profile_hook = lambda: mod._hook
    sys.modules["antenv.axon_hooks"] = mod
    try:
        import antenv
        antenv.axon_hooks = mod
        from trn_agent_boot.trn_boot import _ntff_profile_via_ctypes
        mod.set_axon_ntff_profile_hook(_ntff_profile_via_ctypes("/opt/axon/libaxon_pjrt.so"))
    except Exception:
        pass


def run_on_hw(inp, trace=False):
    """Run the kernel on 8 cores. Returns (full output [4,64,16,16,16], results)."""
    from concourse.bass_utils import run_bass_kernel_spmd

    if "nc" not in _CACHED:
        _install_ntff_hook()
        _CACHED["nc"] = _build_bass()
    nc = _CACHED["nc"]

    svt = _svt_sym(inp['sv'])
    Ks = _assemble_kernel_sym(inp)
    WA, W2 = _weight_slabs(Ks)
    gam, bias = _gam_bias(inp['bn_g_s'], inp['bn_g_v'], inp['bias_s'])

    in_maps = []
    for c in range(NCORES):
        b, h = c // 2, c % 2
        c1, c2e = _core_slabs(svt, b, h)
        in_maps.append({
            "in1": c1,
            "in2e": c2e,
            "wa_in": WA, "w2_in": W2,
            "gam_in": gam.reshape(64, 1),
            "bias_in": bias.reshape(64, 1),
        })

    res = run_bass_kernel_spmd(nc, in_maps, core_ids=list(range(NCORES)), trace=trace)

    y = np.zeros((B, CO, 16, 16, 16), np.float32)
    for c in range(NCORES):
        b, h = c // 2, c % 2
        y[b, :, 8 * h:8 * h + 8] = res.results[c]["out"]
    return y, res


def kernel(**inputs) -> np.ndarray:
    y, _ = run_on_hw(inputs, trace=False)
    return y


# revision 11
# speedup vs baseline: 1.1434x; 1.0009x over previous
"""SE(3)-CNN block (TensorProduct -> SE3Conv -> SE3BatchNorm -> BiasRelu) on 8 trn2 cores.

Sharding: core c = (batch b=c//2, out-x-half h=c%2). Each core computes all 64
output channels for 8 of 16 output x-planes of one batch; per-field BN second
moments are combined with a tiny [1,64] AllReduce across all 8 cores.

v2 conv strategy (vs single-plane baseline):
- Every fp32r matmul covers TWO output x-planes = one full psum bank, free
  dim 448-512 (>= 256 keeps fp32r in its fast streaming mode) and half the
  instruction count.
- No M=64 boundary singles: each chunk-1 kx pair block runs its full slot
  range (s=0..9); out-of-range tap contributions land in psum half-banks the
  evacuation never reads (slot 8 L, slot 9) fed from x-padded slab planes.
- chunk2 (channels 128:160, 4 kx taps packed per 128-row contraction) uses
  slots 10-15 (banks 5-7) plus L(0),L(1),L(6),L(7).
- Weight slab dedup: chunk-1 kx blocks stored once in column order
  [0,2 | 4,6 | 1,3 | 5] so every used pair is contiguous (11.2MB vs 24MB).
- Weight DMA on the Activation HWDGE queue, slabs + tail/stat DMAs on SP,
  evacuation adds split across Vector and GpSimd.
"""
import numpy as np
from itertools import product

# problem constants (from spec / reference)
B = 4
S_IN = 16
V_IN = 16
CO = 64          # 16 scalar + 48 vector output channels
CI = 160         # 16 s + 48 v + 96 t_sym
SIZE = 7
PAD = 3
STRIDE = 2
EPS = 1e-5
NCORES = 8
NXS = 23         # x-padded slab planes per core (px 22 feeds garbage slots only)
NZS = 19         # z-padded: zi_slab = zi_global + 2, covering zofs in [-2, 1]
NX2 = 10         # chunk-2 even slab planes (px = 2*xi, xi 0..9)
OXC = 8          # out x-planes per core
PAIRS = [(0, 0), (0, 1), (0, 2), (1, 1), (1, 2), (2, 2)]
VAR_S_DIV = 1.0 / (B * 16 * 16 * 16)
VAR_V_DIV = 1.0 / (B * 3 * 16 * 16 * 16)

SLAB1_SHAPE = (128, NXS, 32, 2, NZS)   # [ci, px, iy, pz, zi]
SLAB2_SHAPE = (128, NX2, 32, 2, NZS)
WA_COLS = 448    # [k0|k2 | k4|k6 | k1|k3 | k5]
W2_COLS = 2 * CO


# ---------------------------------------------------------------- host prep

def _assemble_kernel_sym(inp):
    """Assemble the dense conv kernel [64, 208, 7,7,7] and symmetrize the
    t-block -> [64, 160, 7,7,7]."""
    def blk(w, basis):
        w = np.asarray(w, np.float32)
        basis = np.asarray(basis, np.float32)
        mo, mi, nb = w.shape
        do, di = basis.shape[1], basis.shape[2]
        k = np.einsum('uvb,bijxyz->uivjxyz', w, basis)
        return k.reshape(mo * do, mi * di, SIZE, SIZE, SIZE)

    row_s = np.concatenate([blk(inp['w_ss'], inp['basis_ss']),
                            blk(inp['w_sv'], inp['basis_sv']),
                            blk(inp['w_st'], inp['basis_st'])], axis=1)
    row_v = np.concatenate([blk(inp['w_vs'], inp['basis_vs']),
                            blk(inp['w_vv'], inp['basis_vv']),
                            blk(inp['w_vt'], inp['basis_vt'])], axis=1)
    K = np.concatenate([row_s, row_v], axis=0)  # [64, 208, 7,7,7]

    Ks = np.empty((CO, CI, SIZE, SIZE, SIZE), np.float32)
    Ks[:, :64] = K[:, :64]
    for u in range(16):
        for pi, (i, j) in enumerate(PAIRS):
            src = K[:, 64 + 9 * u + 3 * i + j]
            if i != j:
                src = src + K[:, 64 + 9 * u + 3 * j + i]
            Ks[:, 64 + 6 * u + pi] = src
    return Ks


def _svt_sym(sv):
    """[4,64,32,32,32] -> symmetrized tensor-product features [4,160,32,32,32]."""
    sv = np.asarray(sv, np.float32)
    s = sv[:, :S_IN]
    v = sv[:, S_IN:].reshape(B, V_IN, 3, 32, 32, 32)
    t = np.empty((B, V_IN, 6, 32, 32, 32), np.float32)
    for pi, (i, j) in enumerate(PAIRS):
        t[:, :, pi] = v[:, :, i] * v[:, :, j]
    return np.concatenate([s, v.reshape(B, 48, 32, 32, 32),
                           t.reshape(B, 96, 32, 32, 32)], axis=1)


def _core_slabs(svt, b, h):
    """x/z zero-padded, z-parity-split slabs for core (b, h).

    c1 [128, 23, 32, 2, 19]: plane px holds global ix = px + 16h - 3.
    c2e [128, 10, 32, 2, 19]: block a (rows 32a:32a+32) of plane xi holds
    chunk-2 channels at ix = 2*xi + a + 16h - 3. zi_slab = zi_global + 2.
    """
    sp = svt[b].reshape(CI, 32, 32, 16, 2)   # (ci, x, y, zi, pz); iz = 2*zi + pz
    sp = np.moveaxis(sp, 4, 3)               # (ci, x, y, pz, zi)
    x0 = 16 * h - 3
    c1 = np.zeros(SLAB1_SHAPE, np.float32)
    lo, hi = max(0, x0), min(32, x0 + NXS)
    c1[:, lo - x0:hi - x0, :, :, 2:18] = sp[:128, lo:hi]
    c2e = np.zeros(SLAB2_SHAPE, np.float32)
    for a in range(4):
        for xi in range(NX2):
            ix = 2 * xi + a + x0
            if 0 <= ix < 32:
                c2e[32 * a:32 * a + 32, xi, :, :, 2:18] = sp[128:160, ix]
    return c1, c2e


def _weight_slabs(Ks):
    """(WA, W2). WA [49, 128, 448]: chunk-1 taps in column order
    [k0|k2|k4|k6|k1|k3|k5] (64 cols each); pairs (0,2),(4,6),(1,3) are the
    contiguous 128-col windows at 0, 128, 256; k5 singles at 384.
    W2 [49, 128, 128]: 4-way kx-merged chunk-2 (g=0: kx 0..3 lower 64 cols,
    g=1: kx 4..6 upper)."""
    KXORD = [0, 2, 4, 6, 1, 3, 5]
    WA = np.zeros((49, 128, WA_COLS), np.float32)
    W2 = np.zeros((49, 128, W2_COLS), np.float32)
    for ky, kz in product(range(SIZE), range(SIZE)):
        i = ky * SIZE + kz
        for ci, kx in enumerate(KXORD):
            WA[i, :, 64 * ci:64 * ci + 64] = Ks[:, :128, kx, ky, kz].T
        for g in range(2):
            for a in range(4):
                kx = 4 * g + a
                if kx > 6:
                    continue
                W2[i, 32 * a:32 * a + 32, 64 * g:64 * (g + 1)] = \
                    Ks[:, 128:160, kx, ky, kz].T
    return WA, W2


def _gam_bias(bn_g_s, bn_g_v, bias_s):
    """Per-channel gamma [64] (vector gammas replicated x3) and bias [64]."""
    gam = np.empty(64, np.float32)
    gam[:16] = np.asarray(bn_g_s, np.float32)
    gam[16:] = np.repeat(np.asarray(bn_g_v, np.float32), 3)
    bias = np.zeros(64, np.float32)
    bias[:16] = np.asarray(bias_s, np.float32)
    return gam, bias


# ---------------------------------------------------------------- matmul plan

def _box(ky, kz):
    """Valid output range + slab coords for kernel offsets (ky, kz)."""
    d = kz - 3
    p = d % 2
    zofs = (d - p) // 2
    oy0 = max(0, (4 - ky) // 2)
    oy1 = min(16, (34 - ky) // 2 + 1)
    iy0 = 2 * oy0 + ky - 3
    return dict(p=p, zs=zofs + 2, iy0=iy0, oyc=oy1 - oy0, oy0=oy0)


def _mm_plan():
    """Matmul descriptors (src, i, wc, ww, x0, bank) in issue order.

    Each matmul writes one full psum bank `bank` = slots (2*bank, 2*bank+1):
    partitions 0:ww x 2 halves x [oy0:oy0+oyc] x 16. Slot s holds plane s at
    partitions 0:64 (L) and plane s-1 at 64:128 (U). Chunk-2 c2p uses slots
    10..15 (banks 5-7): slot 10+j-2 L = plane j lower-taps, slot 10+j U =
    plane j upper-taps. Slot 8 L / slot 9 / U(0) are garbage sinks.

    Order: chunk-2 first (its half-size slab loads fast and its compute
    covers the big chunk-1 slab's DMA), then chunk-1.
    """
    plan = []
    for i in range(49):
        plan.append(('c2', i, 0, 64, 0, 0))     # c2s g0 -> L(0),L(1)
        plan.append(('c2', i, 0, 128, 2, 5))    # c2p -> slots (10,11)
        plan.append(('c2', i, 0, 128, 4, 6))    # slots (12,13)
        plan.append(('c2', i, 0, 128, 6, 7))    # slots (14,15)
        plan.append(('c2', i, 64, 64, 8, 3))    # c2s g1 -> L(6),L(7)
    for i in range(49):
        for bank in range(5):                   # P(0,2): px (4b, 4b+2)
            plan.append(('c1', i, 0, 128, 4 * bank, bank))
        for bank in range(5):                   # P(4,6): px (4+4b, 6+4b)
            plan.append(('c1', i, 128, 128, 4 + 4 * bank, bank))
        for bank in range(5):                   # P(1,3): px (1+4b, 3+4b)
            plan.append(('c1', i, 256, 128, 1 + 4 * bank, bank))
        for bank in range(4):                   # S5: px (5+4b, 7+4b), L only
            plan.append(('c1', i, 384, 64, 5 + 4 * bank, bank))
    return plan


_PLAN = _mm_plan()

# stop flags: last matmul touching each psum bank
_LAST_IDX = {}
for _n, _d in enumerate(_PLAN):
    _LAST_IDX[_d[5]] = _n
_STOPS = set(_LAST_IDX.values())


# ---------------------------------------------------------------- numpy shadow

def _shadow_core(c1, c2e, WA, W2):
    """Execute the matmul plan in numpy. Returns conv output [64, 8, 16, 16]."""
    banks = np.zeros((8, 128, 2, 16, 16), np.float32)
    for src, i, wc, ww, x0, bank in _PLAN:
        ky, kz = i // 7, i % 7
        bx = _box(ky, kz)
        sl = c2e if src == 'c2' else c1
        step = 1 if src == 'c2' else 2
        rhs = sl[:, x0:x0 + step + 1:step,
                 bx['iy0']:bx['iy0'] + 2 * bx['oyc']:2, bx['p'],
                 bx['zs']:bx['zs'] + 16]
        lhsT = (W2 if src == 'c2' else WA)[i][:, wc:wc + ww]
        contrib = np.einsum('km,kpbc->mpbc', lhsT, rhs)
        banks[bank][:ww, :, bx['oy0']:bx['oy0'] + bx['oyc'], :] += contrib
    out = np.empty((OXC, CO, 16, 16), np.float32)
    for j in range(OXC):
        acc = banks[j // 2][0:64, j % 2].copy()        # L(j)
        su = j + 1
        acc = acc + banks[su // 2][64:128, su % 2]     # U(j+1)
        if j >= 2:
            s = 8 + j                                  # LB: slot 10+j-2
            acc = acc + banks[s // 2][0:64, s % 2]
        if j <= 5:
            s = 10 + j                                 # UB: slot 10+j
            acc = acc + banks[s // 2][64:128, s % 2]
        out[j] = acc
    return out.transpose(1, 0, 2, 3)


def shadow_forward(inp):
    """Full-model numpy shadow of the device computation (for plan validation)."""
    svt = _svt_sym(inp['sv'])
    Ks = _assemble_kernel_sym(inp)
    WA, W2 = _weight_slabs(Ks)
    gam, bias = _gam_bias(inp['bn_g_s'], inp['bn_g_v'], inp['bias_s'])

    y = np.zeros((B, CO, 16, 16, 16), np.float32)
    ss = np.zeros(64, np.float64)
    for c in range(NCORES):
        b, h = c // 2, c % 2
        c1, c2e = _core_slabs(svt, b, h)
        out = _shadow_core(c1, c2e, WA, W2)
        y[b, :, 8 * h:8 * h + 8] = out
        ss += (out.astype(np.float64) ** 2).sum(axis=(1, 2, 3))

    var = np.empty(64)
    var[:16] = ss[:16] * VAR_S_DIV
    vv = (ss[16::3] + ss[17::3] + ss[18::3]) * VAR_V_DIV
    var[16:] = np.repeat(vv, 3)
    scale = gam / np.sqrt(var + EPS)
    y = y * scale[None, :, None, None, None].astype(np.float32)
    y[:, :16] = np.maximum(y[:, :16] + bias[:16][None, :, None, None, None], 0.0)
    return y


# ---------------------------------------------------------------- bass kernel

_CACHED = {}


def _build_bass():
    import concourse.bass as bass
    import concourse.tile as tile
    import concourse.mybir as mybir
    from concourse import bacc

    f32 = mybir.dt.float32
    f32r = mybir.dt.float32r
    bf16 = mybir.dt.bfloat16

    nc = bacc.Bacc("TRN2", target_bir_lowering=False, debug=False, num_devices=NCORES)

    in1 = nc.dram_tensor("in1", list(SLAB1_SHAPE), f32r, kind="ExternalInput").ap()
    in2e = nc.dram_tensor("in2e", list(SLAB2_SHAPE), f32r, kind="ExternalInput").ap()
    wa_in = nc.dram_tensor("wa_in", [49, 128, WA_COLS], f32r, kind="ExternalInput").ap()
    w2_in = nc.dram_tensor("w2_in", [49, 128, W2_COLS], f32r, kind="ExternalInput").ap()
    gam_in = nc.dram_tensor("gam_in", [64, 1], f32, kind="ExternalInput").ap()
    bias_in = nc.dram_tensor("bias_in", [64, 1], f32, kind="ExternalInput").ap()
    out_d = nc.dram_tensor("out", [CO, OXC, 16, 16], f32, kind="ExternalOutput").ap()

    with tile.TileContext(nc) as tc:
        with (
            tc.tile_pool(name="slab", bufs=1) as slab_pool,
            tc.tile_pool(name="wp", bufs=4) as wpool,
            tc.tile_pool(name="ps", bufs=1, space="PSUM") as ps,
            tc.tile_pool(name="outp", bufs=1) as outp,
            tc.tile_pool(name="stat", bufs=1) as stat,
            tc.tile_pool(name="dram", bufs=1, space="DRAM") as dram,
        ):
            # 8 psum banks = 16 half-bank slots (garbage: 8L, 9, U(0))
            pq = [ps.tile([128, 2, 16, 16], f32, tag=f"pq{t}", name=f"pq{t}")
                  for t in range(8)]

            # tiny BN params first on the SP queue so they're resident early
            gam_t = stat.tile([CO, 1], f32, tag="gam")
            bias_t = stat.tile([CO, 1], f32, tag="bias")
            nc.sync.dma_start(gam_t[:], gam_in[:])
            nc.sync.dma_start(bias_t[:], bias_in[:])

            # slabs in bf16 (fp32r weights keep the contraction accurate; the
            # halved moving-operand footprint relieves SBUF read pressure and
            # halves slab DMA). Split across both HWDGE rings: SP carries the
            # early chunk-2 planes + chunk-1, Activation the late c2 planes.
            sl2 = slab_pool.tile(list(SLAB2_SHAPE), f32r, tag="slab2",
                                 name="slab_c2")
            sl1 = slab_pool.tile(list(SLAB1_SHAPE), f32r, tag="slab",
                                 name="slab_c1")
            nc.sync.dma_start(sl2[:, 0:2], in2e[:, 0:2])
            nc.sync.dma_start(sl2[:, 2:6], in2e[:, 2:6])
            nc.scalar.dma_start(sl2[:, 6:10], in2e[:, 6:10])
            nc.sync.dma_start(sl1[:, 0:12], in1[:, 0:12])
            nc.sync.dma_start(sl1[:, 12:NXS], in1[:, 12:NXS])

            # start=True clears the WHOLE psum bank, so open each bank once
            # with a zero-weight full-bank matmul (also a WAW dep that orders
            # it before every accumulate); all real matmuls use start=False.
            # rhs is a memset zeros tile so the opens run before any DMA lands.
            zw_f = stat.tile([128, 128], f32, tag="zw")
            zr_f = stat.tile([128, 512], f32, tag="zr")
            nc.vector.memset(zw_f[:], 0.0)
            nc.vector.memset(zr_f[:], 0.0)
            zw = zw_f.bitcast(f32r)
            zr = zr_f.bitcast(f32r)
            for t in range(8):
                nc.tensor.matmul(pq[t].rearrange("c a y z -> c (a y z)"),
                                 zw[:], zr[:], start=True, stop=False)

            # weights on the Activation HWDGE queue, one DMA per tile
            w2t = {}
            for i in range(49):
                w = wpool.tile([128, W2_COLS], f32r, tag="w2", name=f"w2_{i}",
                               bufs=4)
                nc.scalar.dma_start(w[:], w2_in[i])
                w2t[i] = w
            wat = {}
            for i in range(49):
                w = wpool.tile([128, WA_COLS], f32r, tag="wa", name=f"wa_{i}",
                               bufs=4)
                nc.scalar.dma_start(w[:], wa_in[i])
                wat[i] = w

            for n, (src, i, wc, ww, x0, bank) in enumerate(_PLAN):
                ky, kz = i // 7, i % 7
                bx = _box(ky, kz)
                if src == 'c2':
                    w, sl, step = w2t[i], sl2, 1
                else:
                    w, sl, step = wat[i], sl1, 2
                rhs = sl[:, x0:x0 + step + 1:step,
                         bx['iy0']:bx['iy0'] + 2 * bx['oyc'] - 1:2, bx['p'],
                         bx['zs']:bx['zs'] + 16]
                out_ap = pq[bank][0:ww, 0:2, bx['oy0']:bx['oy0'] + bx['oyc'], :]
                nc.tensor.matmul(out_ap, w[:, wc:wc + ww], rhs,
                                 start=False, stop=n in _STOPS)

            # evacuate: plane j = L(j) + U(j+1) [+ LB(10+j-2) j>=2]
            #                                   [+ UB(10+j)   j<=5]
            # a TensorTensor may read only ONE psum operand: Scalar engine
            # copies psum->sbuf, Vector accumulates the second psum operand
            osb = outp.tile([CO, OXC, 16, 16], f32, tag="osb")
            usb = outp.tile([128, OXC, 16, 16], f32, tag="usb")
            for j in range(OXC):
                nc.scalar.activation(osb[:, j], pq[j // 2][0:64, j % 2],
                                     mybir.ActivationFunctionType.Copy,
                                     scale=1.0)
                if j >= 2:
                    s = 8 + j
                    nc.vector.tensor_add(osb[:, j], osb[:, j],
                                         pq[s // 2][0:64, s % 2])
                su = j + 1
                nc.scalar.activation(usb[64:128, j],
                                     pq[su // 2][64:128, su % 2],
                                     mybir.ActivationFunctionType.Copy,
                                     scale=1.0)
                if j <= 5:
                    s = 10 + j
                    nc.vector.tensor_add(usb[64:128, j], usb[64:128, j],
                                         pq[s // 2][64:128, s % 2])
            # move upper-half partials down to partitions 0:64 and add
            u_dram = dram.tile([64, OXC, 16, 16], f32, tag="ud")
            nc.sync.dma_start(u_dram[:], usb[64:128])
            nc.sync.dma_start(usb[0:64], u_dram[:])
            of = osb.rearrange("c x y z -> c (x y z)")
            uf = usb.rearrange("c x y z -> c (x y z)")
            nc.vector.tensor_add(of[:, :], of[:, :], uf[0:64, :])

            # per-channel sum of squares in ONE scalar-engine op (Square with
            # free-axis accumulator) -> local variance contribution
            # (linear in the sums, so the AllReduce can carry variance
            # directly and the post-collective chain stays short)
            sq = outp.tile([CO, 2048], f32, tag="sq")
            ssq = stat.tile([CO, 1], f32, tag="ssq")
            nc.scalar.activation(sq[:], of[:, :],
                                 mybir.ActivationFunctionType.Square,
                                 scale=1.0, accum_out=ssq[:, :])
            ss_row = stat.tile([1, 64], f32, tag="ssrow")
            vloc = stat.tile([1, 64], f32, tag="vloc")
            tmp16 = stat.tile([1, 16], f32, tag="tmp16")
            ss_dram = dram.tile([1, 64], f32, tag="ssd")
            nc.sync.dma_start(ss_dram[0, :], ssq[:, 0])
            nc.sync.dma_start(ss_row[:], ss_dram[:])
            nc.vector.tensor_add(tmp16[:], ss_row[:, 16::3], ss_row[:, 17::3])
            nc.vector.tensor_add(tmp16[:], tmp16[:], ss_row[:, 18::3])
            nc.vector.tensor_scalar_mul(vloc[:, 0:16], ss_row[:, 0:16], VAR_S_DIV)
            for j in range(3):
                nc.vector.tensor_scalar_mul(vloc[:, 16 + j::3], tmp16[:], VAR_V_DIV)

            v_dram = dram.tile([1, 64], f32, tag="vd")
            v_red = dram.tile([1, 64], f32, tag="vr")
            nc.sync.dma_start(v_dram[:], vloc[:])
            nc.gpsimd.collective_compute(
                "AllReduce", mybir.AluOpType.add,
                replica_groups=[list(range(NCORES))],
                ins=[v_dram.opt()], outs=[v_red.opt()],
            )

            # scale = gamma / sqrt(var + eps), in per-partition layout
            var_col = stat.tile([CO, 1], f32, tag="varcol")
            nc.sync.dma_start(var_col[:, 0], v_red[0, :])
            eps_t = stat.tile([CO, 1], f32, tag="eps")
            nc.vector.memset(eps_t[:], EPS)
            sd = stat.tile([CO, 1], f32, tag="sd")
            nc.scalar.activation(sd[:], var_col[:], mybir.ActivationFunctionType.Sqrt,
                                 bias=eps_t[:], scale=1.0)
            inv = stat.tile([CO, 1], f32, tag="inv")
            nc.vector.reciprocal(inv[:], sd[:])
            scale_col = stat.tile([CO, 1], f32, tag="sccol")
            nc.vector.tensor_mul(scale_col[:], inv[:], gam_t[:])

            # apply BN scale everywhere, then bias+relu on scalar channels
            nc.vector.tensor_scalar_mul(of[:, :], of[:, :], scale_col[:, :])
            nc.scalar.activation(of[0:16, :], of[0:16, :],
                                 mybir.ActivationFunctionType.Relu,
                                 bias=bias_t[0:16, :], scale=1.0)
            nc.sync.dma_start(out_d[:], osb[:])

    nc.compile()
    return nc


def _install_ntff_hook():
    import sys, types
    if "antenv.axon_hooks" in sys.modules:
        return
    mod = types.ModuleType("antenv.axon_hooks")
    mod._hook = None
    mod.set_axon_ntff_profile_hook = lambda h: setattr(mod, "_hook", h)
    mod.get_axon_ntff_profile_hook = lambda: mod._hook
    sys.modules["antenv.axon_hooks"] = mod
    try:
        import antenv
        antenv.axon_hooks = mod
        from trn_agent_boot.trn_boot import _ntff_profile_via_ctypes
        mod.set_axon_ntff_profile_hook(_ntff_profile_via_ctypes("/opt/axon/libaxon_pjrt.so"))
    except Exception:
        pass


def run_on_hw(inp, trace=False):
    """Run the kernel on 8 cores. Returns (full output [4,64,16,16,16], results)."""
    from concourse.bass_utils import run_bass_kernel_spmd

    if "nc" not in _CACHED:
        _install_ntff_hook()
        _CACHED["nc"] = _build_bass()
    nc = _CACHED["nc"]

    svt = _svt_sym(inp['sv'])
    Ks = _assemble_kernel_sym(inp)
    WA, W2 = _weight_slabs(Ks)
    gam, bias = _gam_bias(inp['bn_g_s'], inp['bn_g_v'], inp['bias_s'])

    in_maps = []
    for c in range(NCORES):
        b, h = c // 2, c % 2
        c1, c2e = _core_slabs(svt, b, h)
        in_maps.append({
            "in1": c1,
            "in2e": c2e,
            "wa_in": WA, "w2_in": W2,
            "gam_in": gam.reshape(64, 1),
            "bias_in": bias.reshape(64, 1),
        })

    res = run_bass_kernel_spmd(nc, in_maps, core_ids=list(range(NCORES)), trace=trace)

    y = np.zeros((B, CO, 16, 16, 16), np.float32)
    for c in range(NCORES):
        b, h = c // 2, c % 2
        y[b, :, 8 * h:8 * h + 8] = res.results[c]["out"]
    return y, res


def kernel(**inputs) -> np.ndarray:
    y, _ = run_on_hw(inputs, trace=False)
    return y
